# revision 2
# baseline (speedup 1.0000x reference)
"""Bradley-Terry loss kernel for Trainium2 — Chebyshev/PE design, v2.

loss = sum_{i!=j} W[i,j] * softplus(b_j - b_i)
     = sum_{m,l} A[m,l] * z[m,l] - ln2 * trace(W),
  z[m,l] = sum_ij W_ij T_m(x_i) T_l(x_j),  x = (b - c)/h in [-1,1]

softplus(h*(y-x)) is approximated by a degree-63 tensor-product Chebyshev
expansion (max abs error ~1e-13 on the beta range), so the whole O(N^2)
contraction is a matmul: per core, TensorE computes
  Y[m, j] = sum_{i in shard} W[i, j] * T_m(x_i)
with the Chebyshev basis C as the stationary operand (double-bf16 hi/lo
columns stacked -> M=128).  W streams in bf16.

v2: the column-side contraction is folded on-device into a tiny dot.
Host precomputes B[m, j] = sum_l A[m, l] T_l(x_j) (= A @ C^T, bf16), and
  total = sum_{m,j} Y[m,j] * B[m,j]
is evaluated by VectorE directly out of PSUM (scalar_tensor_tensor with
accum_out), one [128,1] partial per 512-col slab.  Output shrinks from
4MB (Y) to 8KB (acc) per core, the diag/ln2 correction moves to the
host, and the const loads ride the Activation HWDGE queue so the sync
queue is a pure, immediately-starting W read stream.
"""

import numpy as np
import ml_dtypes

import concourse.bacc as bacc
import concourse.bass as bass
import concourse.mybir as mybir
from concourse import tile
from concourse.bass_utils import run_bass_kernel_spmd

N = 8192
NCORES = 8
R = N // NCORES            # 1024 rows per core
P = 128                    # SBUF partitions
TROWS = R // P             # 8 row-tiles per core
CHALF = 2048               # column group processed per PSUM generation
NHALF = N // CHALF
SLAB = 512                 # PSUM bank free size (fp32)
NSLAB = CHALF // SLAB      # 4 tags x 2 bufs -> 8 PSUM banks
NACC = NHALF * NSLAB       # 16 per-slab partial columns
DEG = 63
M1 = DEG + 1               # 64 chebyshev coefficients
_LN2 = float(np.log(2.0))

_cached_nc = None


def _cheb_vals(x, deg):
    out = np.empty((len(x), deg + 1), dtype=np.float64)
    out[:, 0] = 1.0
    if deg >= 1:
        out[:, 1] = x
    for k in range(2, deg + 1):
        out[:, k] = 2 * x * out[:, k - 1] - out[:, k - 2]
    return out


def _cheb2d_coeffs(f, deg):
    n = deg + 1
    theta = (np.arange(n) + 0.5) * np.pi / n
    pts = np.cos(theta)
    F = f(pts[:, None], pts[None, :])
    Tm = np.cos(np.outer(np.arange(n), theta))
    A = (2.0 / n) * Tm @ F @ ((2.0 / n) * Tm).T
    A[0, :] /= 2
    A[:, 0] /= 2
    return A


def _build():
    nc = bacc.Bacc(
        "TRN2",
        target_bir_lowering=False,
        debug=False,
        enable_asserts=False,
        num_devices=NCORES,
    )
    f32 = mybir.dt.float32
    bf16 = mybir.dt.bfloat16
    w = nc.dram_tensor("w", [R, N], f32, kind="ExternalInput")
    crows = nc.dram_tensor("crows", [P, TROWS * P], bf16, kind="ExternalInput")
    bmat = nc.dram_tensor("bmat", [P, N], bf16, kind="ExternalInput")
    acc = nc.dram_tensor("acc", [P, NACC], f32, kind="ExternalOutput")

    with tile.TileContext(nc) as tc:
        with (
            tc.tile_pool(name="consts", bufs=1) as consts,
            tc.tile_pool(name="wpool", bufs=6) as wpool,
            tc.tile_pool(name="wbpool", bufs=3) as wbpool,
            tc.tile_pool(name="scrpool", bufs=2) as scrpool,
            tc.tile_pool(name="psum", bufs=2, space="PSUM") as pspool,
        ):
            # consts ride the Activation HWDGE queue; sync stays a pure
            # W-read stream that can start at instruction 0
            crows_sb = consts.tile([P, TROWS * P], bf16)
            nc.scalar.dma_start(crows_sb[:], crows.ap())
            bmat_sb = consts.tile([P, N], bf16)
            nc.scalar.dma_start(bmat_sb[:], bmat.ap())
            acc_sb = consts.tile([P, NACC], f32)

            for ch in range(NHALF):
                ps = [
                    pspool.tile([P, SLAB], f32, tag=f"ps{s}", name=f"ps{s}_{ch}")
                    for s in range(NSLAB)
                ]
                for t in range(TROWS):
                    wt = wpool.tile([P, CHALF], f32, tag="w")
                    nc.sync.dma_start(
                        wt[:],
                        w.ap()[t * P : (t + 1) * P, ch * CHALF : (ch + 1) * CHALF],
                    )
                    wb = wbpool.tile([P, CHALF], bf16, tag="wb")
                    nc.vector.tensor_copy(wb[:], wt[:])
                    lhsT = crows_sb[:, t * P : (t + 1) * P]
                    for s in range(NSLAB):
                        nc.tensor.matmul(
                            ps[s][:],
                            lhsT,
                            wb[:, s * SLAB : (s + 1) * SLAB],
                            start=(t == 0),
                            stop=(t == TROWS - 1),
                        )
                # per-slab dot against B, accumulated along the free dim:
                # acc[:, k] = sum_j ps[s][:, j] * B[:, ch*CHALF + s*SLAB + j]
                for s in range(NSLAB):
                    scr = scrpool.tile([P, SLAB], f32, tag="scr")
                    col = ch * NSLAB + s
                    nc.vector.scalar_tensor_tensor(
                        out=scr[:],
                        in0=ps[s][:],
                        scalar=0.0,
                        in1=bmat_sb[:, ch * CHALF + s * SLAB : ch * CHALF + (s + 1) * SLAB],
                        op0=mybir.AluOpType.bypass,
                        op1=mybir.AluOpType.mult,
                        accum_out=acc_sb[:, col : col + 1],
                    )
            nc.scalar.dma_start(acc.ap(), acc_sb[:])

    nc.compile()
    return nc


def _get_nc():
    global _cached_nc
    if _cached_nc is None:
        _cached_nc = _build()
    return _cached_nc


def kernel(win_matrix, betas, _trace=False):
    win_matrix = np.asarray(win_matrix, dtype=np.float32)
    betas = np.asarray(betas, dtype=np.float32)
    nc = _get_nc()

    b64 = betas.astype(np.float64)
    lo, hi = float(b64.min()), float(b64.max())
    c = 0.5 * (lo + hi)
    h = max(0.5 * (hi - lo) * 1.000001, 1e-12)
    x = (b64 - c) / h
    A = _cheb2d_coeffs(lambda X, Y: np.logaddexp(0.0, h * (Y - X)), DEG)
    C = _cheb_vals(x, DEG)                       # [N, 64] f64
    C_hi = C.astype(ml_dtypes.bfloat16)
    C_lo = (C - C_hi.astype(np.float64)).astype(ml_dtypes.bfloat16)

    # B[m, j] = sum_l A[m, l] T_l(x_j); stacked twice so it aligns with
    # the hi/lo-stacked Y rows (Y_full = Y[:64] + Y[64:]).
    B = A @ C.T                                  # [64, N] f64
    bmat_np = np.ascontiguousarray(
        np.concatenate([B, B], axis=0).astype(ml_dtypes.bfloat16)
    )

    in_maps = []
    for cc in range(NCORES):
        rows = slice(cc * R, (cc + 1) * R)
        stacked = np.concatenate(
            [C_hi[rows].reshape(TROWS, P, M1), C_lo[rows].reshape(TROWS, P, M1)],
            axis=2,
        )  # [t, p, 128]
        crows_np = np.ascontiguousarray(
            stacked.transpose(1, 0, 2).reshape(P, TROWS * P)
        )
        in_maps.append(
            {
                "w": np.ascontiguousarray(win_matrix[rows]),
                "crows": crows_np,
                "bmat": bmat_np,
            }
        )
    res = run_bass_kernel_spmd(
        nc, in_maps, core_ids=list(range(NCORES)), trace=_trace
    )

    total = 0.0
    for cc in range(NCORES):
        total += float(res.results[cc]["acc"].astype(np.float64).sum())
    total -= _LN2 * float(np.trace(win_matrix.astype(np.float64)))
    if _trace:
        kernel.last_results = res
    return np.array(total, dtype=np.float32)


# revision 4
# speedup vs baseline: 1.0125x; 1.0125x over previous
"""Bradley-Terry loss kernel for Trainium2 — Chebyshev/PE design, v2.

loss = sum_{i!=j} W[i,j] * softplus(b_j - b_i)
     = sum_{m,l} A[m,l] * z[m,l] - ln2 * trace(W),
  z[m,l] = sum_ij W_ij T_m(x_i) T_l(x_j),  x = (b - c)/h in [-1,1]

softplus(h*(y-x)) is approximated by a degree-63 tensor-product Chebyshev
expansion (max abs error ~1e-13 on the beta range), so the whole O(N^2)
contraction is a matmul: per core, TensorE computes
  Y[m, j] = sum_{i in shard} W[i, j] * T_m(x_i)
with the Chebyshev basis C as the stationary operand (double-bf16 hi/lo
columns stacked -> M=128).  W streams in bf16.

v2: the column-side contraction is folded on-device into a tiny dot.
Host precomputes B[m, j] = sum_l A[m, l] T_l(x_j) (= A @ C^T, bf16), and
  total = sum_{m,j} Y[m,j] * B[m,j]
is evaluated by VectorE directly out of PSUM (scalar_tensor_tensor with
accum_out), one [128,1] partial per 512-col slab.  Output shrinks from
4MB (Y) to 8KB (acc) per core, the diag/ln2 correction moves to the
host, and the const loads ride the Activation HWDGE queue so the sync
queue is a pure, immediately-starting W read stream.
"""

import numpy as np
import ml_dtypes

import concourse.bacc as bacc
import concourse.bass as bass
import concourse.mybir as mybir
from concourse import tile
from concourse.bass_utils import run_bass_kernel_spmd

N = 8192
NCORES = 8
R = N // NCORES            # 1024 rows per core
P = 128                    # SBUF partitions
TROWS = R // P             # 8 row-tiles per core
CHALF = 2048               # column group processed per PSUM generation
NHALF = N // CHALF
SLAB = 512                 # PSUM bank free size (fp32)
NSLAB = CHALF // SLAB      # 4 tags x 2 bufs -> 8 PSUM banks
NACC = NHALF * NSLAB       # 16 per-slab partial columns
DEG = 63
M1 = DEG + 1               # 64 chebyshev coefficients
_LN2 = float(np.log(2.0))

_cached_nc = None


def _cheb_vals(x, deg):
    out = np.empty((len(x), deg + 1), dtype=np.float64)
    out[:, 0] = 1.0
    if deg >= 1:
        out[:, 1] = x
    for k in range(2, deg + 1):
        out[:, k] = 2 * x * out[:, k - 1] - out[:, k - 2]
    return out


def _cheb2d_coeffs(f, deg):
    n = deg + 1
    theta = (np.arange(n) + 0.5) * np.pi / n
    pts = np.cos(theta)
    F = f(pts[:, None], pts[None, :])
    Tm = np.cos(np.outer(np.arange(n), theta))
    A = (2.0 / n) * Tm @ F @ ((2.0 / n) * Tm).T
    A[0, :] /= 2
    A[:, 0] /= 2
    return A


def _build():
    nc = bacc.Bacc(
        "TRN2",
        target_bir_lowering=False,
        debug=False,
        enable_asserts=False,
        num_devices=NCORES,
    )
    f32 = mybir.dt.float32
    bf16 = mybir.dt.bfloat16
    w = nc.dram_tensor("w", [R, N], f32, kind="ExternalInput")
    crows = nc.dram_tensor("crows", [P, TROWS * P], bf16, kind="ExternalInput")
    bmat = nc.dram_tensor("bmat", [P, N], bf16, kind="ExternalInput")
    acc = nc.dram_tensor("acc", [P, NACC], f32, kind="ExternalOutput")

    with tile.TileContext(nc) as tc:
        with (
            tc.tile_pool(name="consts", bufs=1) as consts,
            tc.tile_pool(name="wpool", bufs=8) as wpool,
            tc.tile_pool(name="wbpool", bufs=3) as wbpool,
            tc.tile_pool(name="scrpool", bufs=2) as scrpool,
            tc.tile_pool(name="psum", bufs=2, space="PSUM") as pspool,
        ):
            # consts ride the Activation HWDGE queue; sync stays a pure
            # W-read stream that can start at instruction 0
            crows_sb = consts.tile([P, TROWS * P], bf16)
            nc.scalar.dma_start(crows_sb[:], crows.ap())
            bmat_sb = consts.tile([P, N], bf16)
            nc.scalar.dma_start(bmat_sb[:], bmat.ap())
            acc_sb = consts.tile([P, NACC], f32)

            # per-slab dot against B, accumulated along the free dim:
            # acc[:, k] = sum_j ps[s][:, j] * B[:, ch*CHALF + s*SLAB + j]
            def emit_dot(ps_s, ch, s):
                scr = scrpool.tile([P, SLAB], f32, tag="scr")
                col = ch * NSLAB + s
                nc.vector.scalar_tensor_tensor(
                    out=scr[:],
                    in0=ps_s[:],
                    scalar=0.0,
                    in1=bmat_sb[:, ch * CHALF + s * SLAB : ch * CHALF + (s + 1) * SLAB],
                    op0=mybir.AluOpType.bypass,
                    op1=mybir.AluOpType.mult,
                    accum_out=acc_sb[:, col : col + 1],
                )

            # dots for chunk ch are deferred into chunk ch+1's tile loop:
            # DVE is strict FIFO, so a dot emitted right after its chunk's
            # last matmul would block the next chunk's casts behind it,
            # stalling PE and backing up the W stream at every boundary.
            pending = None
            for ch in range(NHALF):
                ps = [
                    pspool.tile([P, SLAB], f32, tag=f"ps{s}", name=f"ps{s}_{ch}")
                    for s in range(NSLAB)
                ]
                for t in range(TROWS):
                    wt = wpool.tile([P, CHALF], f32, tag="w")
                    nc.sync.dma_start(
                        wt[:],
                        w.ap()[t * P : (t + 1) * P, ch * CHALF : (ch + 1) * CHALF],
                    )
                    wb = wbpool.tile([P, CHALF], bf16, tag="wb")
                    nc.vector.tensor_copy(wb[:], wt[:])
                    if pending is not None and 1 <= t <= NSLAB:
                        emit_dot(pending[t - 1], ch - 1, t - 1)
                    lhsT = crows_sb[:, t * P : (t + 1) * P]
                    for s in range(NSLAB):
                        nc.tensor.matmul(
                            ps[s][:],
                            lhsT,
                            wb[:, s * SLAB : (s + 1) * SLAB],
                            start=(t == 0),
                            stop=(t == TROWS - 1),
                        )
                pending = ps
            for s in range(NSLAB):
                emit_dot(pending[s], NHALF - 1, s)
            nc.scalar.dma_start(acc.ap(), acc_sb[:])

    nc.compile()
    return nc


def _get_nc():
    global _cached_nc
    if _cached_nc is None:
        _cached_nc = _build()
    return _cached_nc


def kernel(win_matrix, betas, _trace=False):
    win_matrix = np.asarray(win_matrix, dtype=np.float32)
    betas = np.asarray(betas, dtype=np.float32)
    nc = _get_nc()

    b64 = betas.astype(np.float64)
    lo, hi = float(b64.min()), float(b64.max())
    c = 0.5 * (lo + hi)
    h = max(0.5 * (hi - lo) * 1.000001, 1e-12)
    x = (b64 - c) / h
    A = _cheb2d_coeffs(lambda X, Y: np.logaddexp(0.0, h * (Y - X)), DEG)
    C = _cheb_vals(x, DEG)                       # [N, 64] f64
    C_hi = C.astype(ml_dtypes.bfloat16)
    C_lo = (C - C_hi.astype(np.float64)).astype(ml_dtypes.bfloat16)

    # B[m, j] = sum_l A[m, l] T_l(x_j); stacked twice so it aligns with
    # the hi/lo-stacked Y rows (Y_full = Y[:64] + Y[64:]).
    B = A @ C.T                                  # [64, N] f64
    bmat_np = np.ascontiguousarray(
        np.concatenate([B, B], axis=0).astype(ml_dtypes.bfloat16)
    )

    in_maps = []
    for cc in range(NCORES):
        rows = slice(cc * R, (cc + 1) * R)
        stacked = np.concatenate(
            [C_hi[rows].reshape(TROWS, P, M1), C_lo[rows].reshape(TROWS, P, M1)],
            axis=2,
        )  # [t, p, 128]
        crows_np = np.ascontiguousarray(
            stacked.transpose(1, 0, 2).reshape(P, TROWS * P)
        )
        in_maps.append(
            {
                "w": np.ascontiguousarray(win_matrix[rows]),
                "crows": crows_np,
                "bmat": bmat_np,
            }
        )
    res = run_bass_kernel_spmd(
        nc, in_maps, core_ids=list(range(NCORES)), trace=_trace
    )

    total = 0.0
    for cc in range(NCORES):
        total += float(res.results[cc]["acc"].astype(np.float64).sum())
    total -= _LN2 * float(np.trace(win_matrix.astype(np.float64)))
    if _trace:
        kernel.last_results = res
    return np.array(total, dtype=np.float32)
